# revision 38
# baseline (speedup 1.0000x reference)
"""Multi-head self-attention (B=4, N=2048, C=768, H=12, D=64) on 8 TRN2 NeuronCores.

Sharding: (batch, head-group) — core c handles batch c//2, heads (c%2)*6..(c%2)*6+5.
Each core computes its 6 heads' attention plus the partial output projection;
the host sums the two partials per batch and adds the bias terms.

v2: ACT-saturated pipeline. The exp() stream (192 x [128,1024] activations,
~1.08us each) is the kernel floor; everything else rides in its slack.
  - scores: K=64 MMs straight from the pair-merged qt2/kt2 tiles (head 2t on
    partitions 0:64, head 2t+1 on 64:128) through a 2-slot PSUM round-robin;
    exp alternates heads A/B so each psum slot is rewritten while the other
    head's exp runs.
  - attnV: two [128,512] psum accumulators (1 bank each), q-halves and
    m-halves time-multiplexed; DVE drains/adds assemble un[h] in SBUF.
  - phase 1 (QKV proj) and phase 3 (out proj) stream through 2 dedicated
    filler psum banks in PE slack; passes are software-pipelined so the
    next pass's first scores issue before the current pass's trailing burst.
"""

import numpy as np
import ml_dtypes

B, N, C = 4, 2048, 768
H, D = 12, 64
SCALE = D ** -0.5
HL = 6            # heads per core
QK = HL * D       # 384, width of q (= k = v) section per core
KS = 7            # K subtiles (896 = 7*128 rows incl ones/bias row + pad)
P = 128
NT = N            # tokens
NCH = 4           # token chunks of 512
SC = 512
MT = N // P       # 16 token tiles
HW = 1024         # q-tokens per pass (u-half)

_cache = {}


def _build():
    import concourse.bass as bass
    import concourse.mybir as mybir
    import concourse.tile as tile
    from concourse import bacc

    f32 = mybir.dt.float32
    bf16 = mybir.dt.bfloat16

    nc = bacc.Bacc(None, target_bir_lowering=False)
    xt_d = nc.declare_dram_parameter("xt", [KS * P, NT], bf16, isOutput=False)
    wq_d = nc.declare_dram_parameter("wq", [KS * P, 3 * QK], bf16, isOutput=False)
    wp_d = nc.declare_dram_parameter("wp", [QK, C], bf16, isOutput=False)
    bias_d = nc.declare_dram_parameter("bias_qk", [P, 2 * QK // P], f32, isOutput=False)
    out_d = nc.declare_dram_parameter("out", [NT, C], f32, isOutput=True)

    xt_r = xt_d.rearrange("(o p) n -> p o n", p=P)
    wq_r = wq_d.rearrange("(o p) n -> p o n", p=P)
    wp_r = wp_d.rearrange("(o p) n -> p o n", p=P)

    with tile.TileContext(nc) as tc:
        with (
            tc.tile_pool(name="persist", bufs=1) as persist,
            tc.tile_pool(name="e_pool", bufs=14) as e_pool,
            tc.tile_pool(name="un_pool", bufs=3) as un_pool,
            tc.tile_pool(name="rec_pool", bufs=4) as rec_pool,
            tc.tile_pool(name="bc_pool", bufs=4) as bc_pool,
            tc.tile_pool(name="stage_pool", bufs=3) as stage_pool,
            tc.tile_pool(name="dr", bufs=4, space="DRAM") as dr_pool,
            tc.tile_pool(name="psS", bufs=2, space="PSUM") as psS,
            tc.tile_pool(name="psO", bufs=3, space="PSUM") as psO,
            tc.tile_pool(name="psF", bufs=1, space="PSUM") as psF,
        ):
            xt = persist.tile([P, KS, NT], bf16)
            wq = persist.tile([P, KS, 3 * QK], bf16)
            wp = persist.tile([P, QK // P, C], bf16)
            # pair-merged Q^T/K^T: head 2t at partitions 0:64, 2t+1 at 64:128
            qt2 = [persist.tile([P, NT], bf16, name=f"qt2_{t}") for t in range(3)]
            kt2 = [persist.tile([P, NT], bf16, name=f"kt2_{t}") for t in range(3)]
            vv = persist.tile([P, MT, HL, P], bf16)     # V_aug per token-tile/head
            # out^T, split per u-half so proj deps stay clean:
            # outt[o][u] holds partitions o*128.. of out^T for tokens u*1024..
            outt = [[persist.tile([P, HW], bf16, name=f"outt{o}_{u}")
                     for u in range(2)] for o in range(QK // P)]
            bias_qk = persist.tile([P, 2 * QK // P], f32)

            eng = [nc.sync, nc.scalar, nc.gpsimd]
            nc.sync.dma_start(bias_qk[:, :], bias_d[:, :])
            # first-needed loads (wq q-section + xt j=0) first, alternating queues
            for o in range(KS - 1):
                eng[o % 2].dma_start(wq[:, o, 0:QK], wq_r[:, o, 0:QK])
                eng[(o + 1) % 2].dma_start(xt[:, o, 0:SC], xt_r[:, o, 0:SC])
            for j in range(1, NCH):
                for o in range(KS - 1):
                    eng[(j * KS + o) % 2].dma_start(
                        xt[:, o, j * SC:(j + 1) * SC], xt_r[:, o, j * SC:(j + 1) * SC]
                    )
            for o in range(KS - 1):
                nc.gpsimd.dma_start(wq[:, o, QK:2 * QK], wq_r[:, o, QK:2 * QK])
            for o in range(KS - 1):
                nc.gpsimd.dma_start(wq[:, o, 2 * QK:3 * QK], wq_r[:, o, 2 * QK:3 * QK])
            for o in range(QK // P):
                nc.gpsimd.dma_start(wp[:, o, :], wp_r[:, o, :])

            # V_aug col layout (per head):
            #   even heads: [v(64) | ones(32) | zeros(32)]
            #   odd  heads: [zeros(32) | ones(32) | v(64)]
            for h in range(HL):
                nc.vector.memset(vv[:, :, h, 32:96] if h % 2 else vv[:, :, h, 64:96], 1.0)
                nc.vector.memset(vv[:, :, h, 0:32] if h % 2 else vv[:, :, h, 96:128], 0.0)

            # ---------------- phase-1/3 units (filler banks) ----------------
            def qkt_chunk(mi, j, pool=None, tag="fps"):
                # tokens j*512.. of [Q^T;K^T] dim-chunk mi (mi<3 -> Q)
                dst = (qt2 if mi < 3 else kt2)[mi % 3]
                pl = pool or psF
                ps = pl.tile([P, SC], f32, tag=tag, name="ps_f")
                for o in range(KS - 1):
                    nc.tensor.matmul(
                        ps[:, :SC],
                        lhsT=wq[:, o, mi * P:(mi + 1) * P],
                        rhs=xt[:, o, j * SC:(j + 1) * SC],
                        start=(o == 0),
                        stop=(o == KS - 2),
                    )
                sc = slice(j * SC, (j + 1) * SC)
                nc.vector.tensor_scalar_add(
                    out=dst[:, sc], in0=ps[:, :SC], scalar1=bias_qk[:, mi:mi + 1])

            def v_mtile(ti, pool=None, tag="fps"):
                pl = pool or psF
                ps = pl.tile([P, SC], f32, tag=tag, name="ps_f")
                for o in range(KS - 1):
                    nc.tensor.matmul(
                        ps[:, :QK],
                        lhsT=xt[:, o, ti * P:(ti + 1) * P],
                        rhs=wq[:, o, 2 * QK:3 * QK],
                        start=(o == 0),
                        stop=(o == KS - 2),
                    )
                psv = ps[:, :QK].rearrange("p (h d) -> p h d", h=HL)
                nc.vector.tensor_copy(out=vv[:, ti, 0:HL:2, 0:64], in_=psv[:, 0:HL:2, :])
                nc.vector.tensor_copy(out=vv[:, ti, 1:HL:2, 64:128], in_=psv[:, 1:HL:2, :])

            def proj(ti, pool=None, tag="fps"):
                # token tile ti -> out rows; reads outt[o][ti//8]
                u = ti // 8
                tl = ti % 8
                stage = stage_pool.tile([P, C], f32)
                for (w0, wn) in ((0, 512), (512, 256)):
                    ps = (pool or psF).tile([P, wn], f32, tag=tag, name="ps_pj")
                    for o in range(QK // P):
                        nc.tensor.matmul(
                            ps[:, :wn],
                            lhsT=outt[o][u][:, tl * P:(tl + 1) * P],
                            rhs=wp[:, o, w0:w0 + wn],
                            start=(o == 0),
                            stop=(o == QK // P - 1),
                        )
                    nc.vector.tensor_copy(out=stage[:, w0:w0 + wn], in_=ps[:, :wn])
                nc.sync.dma_start(out_d[ti * P:(ti + 1) * P, :], stage[:, :])

            urgent = []
            lazy = []

            def flr():
                if urgent:
                    urgent.pop(0)()
                elif lazy:
                    lazy.pop(0)()

            # ---------------- phase-2 pass machinery ----------------
            def make_state(t, u):
                st = {"t": t, "u": u, "e": {}}

                def spair(m):
                    for hi in (0, 1):
                        bp = hi * 64
                        ps = psS.tile([P, HW], f32, tag="ps", name="ps_s")
                        for jj in (0, 1):
                            nc.tensor.matmul(
                                ps[:, jj * SC:(jj + 1) * SC],
                                lhsT=kt2[t][bp:bp + 64, m * P:(m + 1) * P],
                                rhs=qt2[t][bp:bp + 64,
                                           u * HW + jj * SC:u * HW + (jj + 1) * SC],
                                start=True,
                                stop=True,
                            )
                        e = e_pool.tile([P, HW], bf16, tag="e", name="e")
                        nc.scalar.activation(
                            e[:, :], ps[:, :], mybir.ActivationFunctionType.Exp,
                            scale=float(SCALE),
                        )
                        st["e"][(hi, m)] = e

                st["spair"] = spair
                return st

            QG = 4  # m-tiles per attnV accumulation group

            def attn_mm(st, hi, qh, m, po_t):
                h = 2 * st["t"] + hi
                nc.tensor.matmul(
                    po_t[:, :],
                    lhsT=vv[:, m, h, :],
                    rhs=st["e"][(hi, m)][:, qh * SC:(qh + 1) * SC],
                    start=(m % QG == 0),
                    stop=(m % QG == QG - 1),
                )

            def drain(un_t, qh, po_t, first):
                dst = un_t[:, qh * SC:(qh + 1) * SC]
                if first:
                    nc.vector.tensor_copy(out=dst, in_=po_t[:, :])
                else:
                    nc.vector.tensor_add(dst, po_t[:, :], dst)

            eng_n = [nc.sync, nc.gpsimd]  # normalize DMAs: never the ACT queue

            def normalize(st, hi, un_t, nck=1, cks=None):
                t, u = st["t"], st["u"]
                h = 2 * t + hi
                po = hi * 64
                dlane = 64 if hi == 0 else 32
                cw = HW // nck
                for ck in (range(nck) if cks is None else cks):
                    lo = ck * cw
                    # reciprocal of the denominator row spread over 128 lanes:
                    # row -> DRAM -> [128, cw/128] -> recip -> DRAM -> bcast load
                    dn = dr_pool.tile([1, cw], f32, name="dn", tag="dn")
                    eng_n[(h + 0) % 2].dma_start(dn[:, :], un_t[dlane:dlane + 1, lo:lo + cw])
                    dnp = rec_pool.tile([P, cw // P], f32, name="dnp", tag="dnp")
                    eng_n[(h + 1) % 2].dma_start(dnp[:, :], dn[0].rearrange("(p f) -> p f", p=P))
                    rcp = rec_pool.tile([P, cw // P], f32, name="rcp", tag="rcp")
                    nc.vector.reciprocal(rcp[:, :], dnp[:, :])
                    rd = dr_pool.tile([1, cw], f32, name="rd", tag="rd")
                    eng_n[(h + 0) % 2].dma_start(rd[0].rearrange("(p f) -> p f", p=P), rcp[:, :])
                    bc = bc_pool.tile([P, cw], f32, name="bc", tag="bc")
                    eng_n[(h + 1) % 2].dma_start(
                        bc[:, :],
                        bass.AP(tensor=rd.tensor, offset=rd.offset, ap=[[0, P]] + list(rd.ap)),
                    )
                    nc.vector.tensor_mul(
                        outt[t][u][po:po + 64, lo:lo + cw],
                        un_t[po:po + 64, lo:lo + cw], bc[po:po + 64, :]
                    )

            def run_pass(st, next_st=None, nck=1):
                unA = un_pool.tile([P, HW], f32, tag="un", name="unA")
                unB = un_pool.tile([P, HW], f32, tag="un", name="unB")
                poA = poB = None
                for m in range(MT):
                    if m % QG == 0:
                        poA = psO.tile([P, SC], f32, tag="po", name="poA")
                        poB = psO.tile([P, SC], f32, tag="po", name="poB")
                    if m == MT - 1 and next_st is not None:
                        next_st["spair"](0)
                    attn_mm(st, 0, 0, m, poA)
                    attn_mm(st, 1, 0, m, poB)
                    if m < MT - 2:
                        st["spair"](m + 2)
                    elif m == MT - 1 and next_st is not None:
                        next_st["spair"](1)
                    if m % QG == QG - 1:
                        q = m // QG
                        drain(unA, 0, poA, first=(q == 0))
                        drain(unB, 0, poB, first=(q == 0))
                        pqA = psO.tile([P, SC], f32, tag="po", name="poA")
                        pqB = psO.tile([P, SC], f32, tag="po", name="poB")
                        for mm in range(q * QG, (q + 1) * QG):
                            attn_mm(st, 0, 1, mm, pqA)
                        for mm in range(q * QG, (q + 1) * QG):
                            attn_mm(st, 1, 1, mm, pqB)
                        drain(unA, 1, pqA, first=(q == 0))
                        drain(unB, 1, pqB, first=(q == 0))
                    flr()
                if next_st is None:
                    # final pass: interleave the two normalize chunks with the
                    # two proj waves so proj PE work hides the chain latency
                    normalize(st, 0, unA, nck=2, cks=(0,))
                    normalize(st, 1, unB, nck=2, cks=(0,))
                    for ti in (8, 9, 10, 11):
                        proj(ti, pool=(psS if ti % 2 else psO),
                             tag=("ps" if ti % 2 else "po"))
                    normalize(st, 0, unA, nck=2, cks=(1,))
                    normalize(st, 1, unB, nck=2, cks=(1,))
                    for ti in (12, 13, 14, 15):
                        proj(ti, pool=(psS if ti % 2 else psO),
                             tag=("ps" if ti % 2 else "po"))
                else:
                    normalize(st, 0, unA, nck=nck)
                    normalize(st, 1, unB, nck=nck)

            # ---------------- schedule ----------------
            # prologue: inputs for pass (0, u=0), spread across all psum banks
            # (emission order = dependency order; every tile's write must be
            # emitted before its first reader)
            qkt_chunk(0, 0, pool=psS, tag="ps")
            qkt_chunk(3, 0, pool=psS, tag="ps")
            qkt_chunk(0, 1, pool=psF, tag="fps")
            v_mtile(0, pool=psO, tag="po")
            v_mtile(1, pool=psO, tag="po")
            v_mtile(2, pool=psF, tag="fps")
            v_mtile(3, pool=psS, tag="ps")
            v_mtile(4, pool=psS, tag="ps")
            v_mtile(5, pool=psF, tag="fps")

            def Q(mi, j):
                return lambda: qkt_chunk(mi, j)

            def V(ti):
                return lambda: v_mtile(ti)

            # per-pass filler queues, deadline-ordered (1 consumed per slot)
            fillers = [
                [Q(3, 1), V(6), V(7), Q(3, 2), V(8), V(9), Q(0, 2), Q(3, 3),
                 V(10), V(11), Q(0, 3), V(12), V(13), V(14), V(15)],
                [Q(1, 0), Q(1, 1), Q(4, 0), Q(4, 1), Q(4, 2), Q(4, 3)],
                [Q(1, 2), Q(1, 3), Q(2, 0), Q(2, 1), Q(5, 0), Q(5, 1)],
                [Q(5, 2), Q(5, 3), Q(2, 2), Q(2, 3)],
                [],
                [lambda ti=ti: proj(ti) for ti in range(8)],
            ]

            order = [(0, 0), (0, 1), (1, 0), (1, 1), (2, 0), (2, 1)]
            states = [make_state(t, u) for (t, u) in order]
            states[0]["spair"](0)
            states[0]["spair"](1)
            for i, st in enumerate(states):
                nxt = states[i + 1] if i + 1 < len(states) else None
                lazy.extend(fillers[i])
                run_pass(st, next_st=nxt, nck=(2 if nxt is None else 1))
            while urgent or lazy:
                (urgent if urgent else lazy).pop(0)()

    nc.compile()
    return nc


def _prep_inputs(x, qkv_w, qkv_b):
    bf = ml_dtypes.bfloat16
    in_maps = []
    for c in range(8):
        b, hs = c // 2, (c % 2) * HL
        xt = np.zeros((KS * P, NT), dtype=bf)
        xt[0:C, :] = x[b].T.astype(bf)
        xt[C, :] = 1.0
        wq = np.zeros((KS * P, 3 * QK), dtype=bf)
        for s in range(3):  # q, k, v sections
            cols = qkv_w[:, s * C + hs * D: s * C + (hs + HL) * D]
            wq[0:C, s * QK:(s + 1) * QK] = cols.astype(bf)
        wq[C, 0:QK] = qkv_b[hs * D:(hs + HL) * D].astype(bf)
        wq[C, QK:2 * QK] = qkv_b[C + hs * D: C + (hs + HL) * D].astype(bf)
        qk_bias = np.concatenate([
            qkv_b[hs * D:(hs + HL) * D], qkv_b[C + hs * D: C + (hs + HL) * D]
        ]).astype(np.float32)
        in_maps.append({"xt": xt, "wq": wq,
                        "bias_qk": np.ascontiguousarray(qk_bias.reshape(6, P).T)})
    return in_maps


def kernel(x, qkv_w, qkv_b, proj_w, proj_b):
    from concourse.bass_utils import run_bass_kernel_spmd

    x = np.asarray(x, dtype=np.float32)
    qkv_w = np.asarray(qkv_w, dtype=np.float32)
    qkv_b = np.asarray(qkv_b, dtype=np.float32)
    proj_w = np.asarray(proj_w, dtype=np.float32)
    proj_b = np.asarray(proj_b, dtype=np.float32)

    if "nc" not in _cache:
        _cache["nc"] = _build()
    nc = _cache["nc"]

    bf = ml_dtypes.bfloat16
    in_maps = _prep_inputs(x, qkv_w, qkv_b)
    for c in range(8):
        hs = (c % 2) * HL
        in_maps[c]["wp"] = proj_w[hs * D:(hs + HL) * D, :].astype(bf)

    res = run_bass_kernel_spmd(nc, in_maps, core_ids=list(range(8)))
    parts = [res.results[c]["out"].astype(np.float32) for c in range(8)]

    # v-bias contribution (exact, f32) + proj bias, added once per batch
    const_row = qkv_b[2 * C:] @ proj_w + proj_b
    out = np.empty((B, N, C), dtype=np.float32)
    for b in range(B):
        out[b] = parts[2 * b] + parts[2 * b + 1] + const_row
    return out


# revision 39
# speedup vs baseline: 1.0710x; 1.0710x over previous
"""Multi-head self-attention (B=4, N=2048, C=768, H=12, D=64) on 8 TRN2 NeuronCores.

Sharding: (batch, head-group) — core c handles batch c//2, heads (c%2)*6..(c%2)*6+5.
Each core computes its 6 heads' attention plus the partial output projection;
the host sums the two partials per batch and adds the bias terms.

v2: ACT-saturated pipeline. The exp() stream (192 x [128,1024] activations,
~1.08us each) is the kernel floor; everything else rides in its slack.
  - scores: K=64 MMs straight from the pair-merged qt2/kt2 tiles (head 2t on
    partitions 0:64, head 2t+1 on 64:128) through a 2-slot PSUM round-robin;
    exp alternates heads A/B so each psum slot is rewritten while the other
    head's exp runs.
  - attnV: two [128,512] psum accumulators (1 bank each), q-halves and
    m-halves time-multiplexed; DVE drains/adds assemble un[h] in SBUF.
  - phase 1 (QKV proj) and phase 3 (out proj) stream through 2 dedicated
    filler psum banks in PE slack; passes are software-pipelined so the
    next pass's first scores issue before the current pass's trailing burst.
"""

import numpy as np
import ml_dtypes

B, N, C = 4, 2048, 768
H, D = 12, 64
SCALE = D ** -0.5
HL = 6            # heads per core
QK = HL * D       # 384, width of q (= k = v) section per core
KS = 7            # K subtiles (896 = 7*128 rows incl ones/bias row + pad)
P = 128
NT = N            # tokens
NCH = 4           # token chunks of 512
SC = 512
MT = N // P       # 16 token tiles
HW = 1024         # q-tokens per pass (u-half)

_cache = {}


def _build():
    import concourse.bass as bass
    import concourse.mybir as mybir
    import concourse.tile as tile
    from concourse import bacc

    f32 = mybir.dt.float32
    bf16 = mybir.dt.bfloat16

    nc = bacc.Bacc(None, target_bir_lowering=False)
    xt_d = nc.declare_dram_parameter("xt", [KS * P, NT], bf16, isOutput=False)
    wq_d = nc.declare_dram_parameter("wq", [KS * P, 3 * QK], bf16, isOutput=False)
    wp_d = nc.declare_dram_parameter("wp", [QK, C], bf16, isOutput=False)
    bias_d = nc.declare_dram_parameter("bias_qk", [P, 2 * QK // P], f32, isOutput=False)
    out_d = nc.declare_dram_parameter("out", [NT, C], f32, isOutput=True)

    xt_r = xt_d.rearrange("(o p) n -> p o n", p=P)
    wq_r = wq_d.rearrange("(o p) n -> p o n", p=P)
    wp_r = wp_d.rearrange("(o p) n -> p o n", p=P)

    with tile.TileContext(nc) as tc:
        with (
            tc.tile_pool(name="persist", bufs=1) as persist,
            tc.tile_pool(name="e_pool", bufs=14) as e_pool,
            tc.tile_pool(name="un_pool", bufs=4) as un_pool,
            tc.tile_pool(name="rec_pool", bufs=4) as rec_pool,
            tc.tile_pool(name="bc_pool", bufs=4) as bc_pool,
            tc.tile_pool(name="stage_pool", bufs=3) as stage_pool,
            tc.tile_pool(name="dr", bufs=4, space="DRAM") as dr_pool,
            tc.tile_pool(name="psS", bufs=2, space="PSUM") as psS,
            tc.tile_pool(name="psO", bufs=3, space="PSUM") as psO,
            tc.tile_pool(name="psF", bufs=1, space="PSUM") as psF,
        ):
            xt = persist.tile([P, KS, NT], bf16)
            wq = persist.tile([P, KS, 3 * QK], bf16)
            wp = persist.tile([P, QK // P, C], bf16)
            # pair-merged Q^T/K^T: head 2t at partitions 0:64, 2t+1 at 64:128
            qt2 = [persist.tile([P, NT], bf16, name=f"qt2_{t}") for t in range(3)]
            kt2 = [persist.tile([P, NT], bf16, name=f"kt2_{t}") for t in range(3)]
            vv = persist.tile([P, MT, HL, P], bf16)     # V_aug per token-tile/head
            # out^T, split per u-half so proj deps stay clean:
            # outt[o][u] holds partitions o*128.. of out^T for tokens u*1024..
            outt = [[persist.tile([P, HW], bf16, name=f"outt{o}_{u}")
                     for u in range(2)] for o in range(QK // P)]
            bias_qk = persist.tile([P, 2 * QK // P], f32)

            eng = [nc.sync, nc.scalar, nc.gpsimd]
            nc.sync.dma_start(bias_qk[:, :], bias_d[:, :])
            # first-needed loads (wq q-section + xt j=0) first, alternating queues
            for o in range(KS - 1):
                eng[o % 2].dma_start(wq[:, o, 0:QK], wq_r[:, o, 0:QK])
                eng[(o + 1) % 2].dma_start(xt[:, o, 0:SC], xt_r[:, o, 0:SC])
            for j in range(1, NCH):
                for o in range(KS - 1):
                    eng[(j * KS + o) % 2].dma_start(
                        xt[:, o, j * SC:(j + 1) * SC], xt_r[:, o, j * SC:(j + 1) * SC]
                    )
            for o in range(KS - 1):
                nc.gpsimd.dma_start(wq[:, o, QK:2 * QK], wq_r[:, o, QK:2 * QK])
            for o in range(KS - 1):
                nc.gpsimd.dma_start(wq[:, o, 2 * QK:3 * QK], wq_r[:, o, 2 * QK:3 * QK])
            for o in range(QK // P):
                nc.gpsimd.dma_start(wp[:, o, :], wp_r[:, o, :])

            # V_aug col layout (per head):
            #   even heads: [v(64) | ones(32) | zeros(32)]
            #   odd  heads: [zeros(32) | ones(32) | v(64)]
            for h in range(HL):
                nc.vector.memset(vv[:, :, h, 32:96] if h % 2 else vv[:, :, h, 64:96], 1.0)
                nc.vector.memset(vv[:, :, h, 0:32] if h % 2 else vv[:, :, h, 96:128], 0.0)

            # ---------------- phase-1/3 units (filler banks) ----------------
            def qkt_chunk(mi, j, pool=None, tag="fps"):
                # tokens j*512.. of [Q^T;K^T] dim-chunk mi (mi<3 -> Q)
                dst = (qt2 if mi < 3 else kt2)[mi % 3]
                pl = pool or psF
                ps = pl.tile([P, SC], f32, tag=tag, name="ps_f")
                for o in range(KS - 1):
                    nc.tensor.matmul(
                        ps[:, :SC],
                        lhsT=wq[:, o, mi * P:(mi + 1) * P],
                        rhs=xt[:, o, j * SC:(j + 1) * SC],
                        start=(o == 0),
                        stop=(o == KS - 2),
                    )
                sc = slice(j * SC, (j + 1) * SC)
                nc.vector.tensor_scalar_add(
                    out=dst[:, sc], in0=ps[:, :SC], scalar1=bias_qk[:, mi:mi + 1])

            def v_mtile(ti, pool=None, tag="fps"):
                pl = pool or psF
                ps = pl.tile([P, SC], f32, tag=tag, name="ps_f")
                for o in range(KS - 1):
                    nc.tensor.matmul(
                        ps[:, :QK],
                        lhsT=xt[:, o, ti * P:(ti + 1) * P],
                        rhs=wq[:, o, 2 * QK:3 * QK],
                        start=(o == 0),
                        stop=(o == KS - 2),
                    )
                psv = ps[:, :QK].rearrange("p (h d) -> p h d", h=HL)
                nc.vector.tensor_copy(out=vv[:, ti, 0:HL:2, 0:64], in_=psv[:, 0:HL:2, :])
                nc.vector.tensor_copy(out=vv[:, ti, 1:HL:2, 64:128], in_=psv[:, 1:HL:2, :])

            def proj(ti, pool=None, tag="fps"):
                # token tile ti -> out rows; reads outt[o][ti//8]
                u = ti // 8
                tl = ti % 8
                stage = stage_pool.tile([P, C], f32)
                for (w0, wn) in ((0, 512), (512, 256)):
                    ps = (pool or psF).tile([P, wn], f32, tag=tag, name="ps_pj")
                    for o in range(QK // P):
                        nc.tensor.matmul(
                            ps[:, :wn],
                            lhsT=outt[o][u][:, tl * P:(tl + 1) * P],
                            rhs=wp[:, o, w0:w0 + wn],
                            start=(o == 0),
                            stop=(o == QK // P - 1),
                        )
                    nc.vector.tensor_copy(out=stage[:, w0:w0 + wn], in_=ps[:, :wn])
                nc.sync.dma_start(out_d[ti * P:(ti + 1) * P, :], stage[:, :])

            urgent = []
            lazy = []

            def flr():
                if urgent:
                    urgent.pop(0)()
                elif lazy:
                    lazy.pop(0)()

            # ---------------- phase-2 pass machinery ----------------
            def make_state(t, u):
                st = {"t": t, "u": u, "e": {}}

                def spair(m):
                    for hi in (0, 1):
                        bp = hi * 64
                        ps = psS.tile([P, HW], f32, tag="ps", name="ps_s")
                        for jj in (0, 1):
                            nc.tensor.matmul(
                                ps[:, jj * SC:(jj + 1) * SC],
                                lhsT=kt2[t][bp:bp + 64, m * P:(m + 1) * P],
                                rhs=qt2[t][bp:bp + 64,
                                           u * HW + jj * SC:u * HW + (jj + 1) * SC],
                                start=True,
                                stop=True,
                            )
                        e = e_pool.tile([P, HW], bf16, tag="e", name="e")
                        nc.scalar.activation(
                            e[:, :], ps[:, :], mybir.ActivationFunctionType.Exp,
                            scale=float(SCALE),
                        )
                        st["e"][(hi, m)] = e

                st["spair"] = spair
                return st

            QG = 4  # m-tiles per attnV accumulation group

            def attn_mm(st, hi, qh, m, po_t):
                h = 2 * st["t"] + hi
                nc.tensor.matmul(
                    po_t[:, :],
                    lhsT=vv[:, m, h, :],
                    rhs=st["e"][(hi, m)][:, qh * SC:(qh + 1) * SC],
                    start=(m % QG == 0),
                    stop=(m % QG == QG - 1),
                )

            def drain(un_t, qh, po_t, first):
                dst = un_t[:, qh * SC:(qh + 1) * SC]
                if first:
                    nc.vector.tensor_copy(out=dst, in_=po_t[:, :])
                else:
                    nc.vector.tensor_add(dst, po_t[:, :], dst)

            eng_n = [nc.sync, nc.gpsimd]  # normalize DMAs: never the ACT queue

            def normalize(st, hi, un_t, nck=1, cks=None):
                t, u = st["t"], st["u"]
                h = 2 * t + hi
                po = hi * 64
                dlane = 64 if hi == 0 else 32
                cw = HW // nck
                for ck in (range(nck) if cks is None else cks):
                    lo = ck * cw
                    # reciprocal of the denominator row spread over 128 lanes:
                    # row -> DRAM -> [128, cw/128] -> recip -> DRAM -> bcast load
                    dn = dr_pool.tile([1, cw], f32, name="dn", tag="dn")
                    eng_n[(h + 0) % 2].dma_start(dn[:, :], un_t[dlane:dlane + 1, lo:lo + cw])
                    dnp = rec_pool.tile([P, cw // P], f32, name="dnp", tag="dnp")
                    eng_n[(h + 1) % 2].dma_start(dnp[:, :], dn[0].rearrange("(p f) -> p f", p=P))
                    rcp = rec_pool.tile([P, cw // P], f32, name="rcp", tag="rcp")
                    nc.vector.reciprocal(rcp[:, :], dnp[:, :])
                    rd = dr_pool.tile([1, cw], f32, name="rd", tag="rd")
                    eng_n[(h + 0) % 2].dma_start(rd[0].rearrange("(p f) -> p f", p=P), rcp[:, :])
                    bc = bc_pool.tile([P, cw], f32, name="bc", tag="bc")
                    eng_n[(h + 1) % 2].dma_start(
                        bc[:, :],
                        bass.AP(tensor=rd.tensor, offset=rd.offset, ap=[[0, P]] + list(rd.ap)),
                    )
                    nc.vector.tensor_mul(
                        outt[t][u][po:po + 64, lo:lo + cw],
                        un_t[po:po + 64, lo:lo + cw], bc[po:po + 64, :]
                    )

            def run_pass(st, next_st=None, nck=1):
                unA = un_pool.tile([P, HW], f32, tag="un", name="unA")
                unB = un_pool.tile([P, HW], f32, tag="un", name="unB")
                poA = poB = None
                for m in range(MT):
                    if m % QG == 0:
                        poA = psO.tile([P, SC], f32, tag="po", name="poA")
                        poB = psO.tile([P, SC], f32, tag="po", name="poB")
                    if m == MT - 1 and next_st is not None:
                        next_st["spair"](0)
                    attn_mm(st, 0, 0, m, poA)
                    attn_mm(st, 1, 0, m, poB)
                    if m < MT - 2:
                        st["spair"](m + 2)
                    elif m == MT - 1 and next_st is not None:
                        next_st["spair"](1)
                    if m % QG == QG - 1:
                        q = m // QG
                        drain(unA, 0, poA, first=(q == 0))
                        drain(unB, 0, poB, first=(q == 0))
                        pqA = psO.tile([P, SC], f32, tag="po", name="poA")
                        pqB = psO.tile([P, SC], f32, tag="po", name="poB")
                        for mm in range(q * QG, (q + 1) * QG):
                            attn_mm(st, 0, 1, mm, pqA)
                        for mm in range(q * QG, (q + 1) * QG):
                            attn_mm(st, 1, 1, mm, pqB)
                        drain(unA, 1, pqA, first=(q == 0))
                        drain(unB, 1, pqB, first=(q == 0))
                    flr()
                if next_st is None:
                    # final pass: interleave the two normalize chunks with the
                    # two proj waves so proj PE work hides the chain latency
                    normalize(st, 0, unA, nck=2, cks=(0,))
                    normalize(st, 1, unB, nck=2, cks=(0,))
                    for ti in (8, 9, 10, 11):
                        proj(ti, pool=(psS if ti % 2 else psO),
                             tag=("ps" if ti % 2 else "po"))
                    normalize(st, 0, unA, nck=2, cks=(1,))
                    normalize(st, 1, unB, nck=2, cks=(1,))
                    for ti in (12, 13, 14, 15):
                        proj(ti, pool=(psS if ti % 2 else psO),
                             tag=("ps" if ti % 2 else "po"))
                else:
                    normalize(st, 0, unA, nck=nck)
                    normalize(st, 1, unB, nck=nck)

            # ---------------- schedule ----------------
            # prologue: inputs for pass (0, u=0), spread across all psum banks
            # (emission order = dependency order; every tile's write must be
            # emitted before its first reader)
            qkt_chunk(0, 0, pool=psS, tag="ps")
            qkt_chunk(3, 0, pool=psS, tag="ps")
            qkt_chunk(0, 1, pool=psF, tag="fps")
            v_mtile(0, pool=psO, tag="po")
            v_mtile(1, pool=psO, tag="po")
            v_mtile(2, pool=psF, tag="fps")
            v_mtile(3, pool=psS, tag="ps")
            v_mtile(4, pool=psS, tag="ps")
            v_mtile(5, pool=psF, tag="fps")

            def Q(mi, j):
                return lambda: qkt_chunk(mi, j)

            def V(ti):
                return lambda: v_mtile(ti)

            # per-pass filler queues, deadline-ordered (1 consumed per slot)
            fillers = [
                [Q(3, 1), V(6), V(7), Q(3, 2), V(8), V(9), Q(0, 2), Q(3, 3),
                 V(10), V(11), Q(0, 3), V(12), V(13), V(14), V(15)],
                [Q(1, 0), Q(1, 1), Q(4, 0), Q(4, 1), Q(4, 2), Q(4, 3)],
                [Q(1, 2), Q(1, 3), Q(2, 0), Q(2, 1), Q(5, 0), Q(5, 1)],
                [Q(5, 2), Q(5, 3), Q(2, 2), Q(2, 3)],
                [],
                [lambda ti=ti: proj(ti) for ti in range(8)],
            ]

            order = [(0, 0), (0, 1), (1, 0), (1, 1), (2, 0), (2, 1)]
            states = [make_state(t, u) for (t, u) in order]
            states[0]["spair"](0)
            states[0]["spair"](1)
            for i, st in enumerate(states):
                nxt = states[i + 1] if i + 1 < len(states) else None
                lazy.extend(fillers[i])
                run_pass(st, next_st=nxt, nck=(2 if nxt is None else 1))
            while urgent or lazy:
                (urgent if urgent else lazy).pop(0)()

    nc.compile()
    return nc


def _prep_inputs(x, qkv_w, qkv_b):
    bf = ml_dtypes.bfloat16
    in_maps = []
    for c in range(8):
        b, hs = c // 2, (c % 2) * HL
        xt = np.zeros((KS * P, NT), dtype=bf)
        xt[0:C, :] = x[b].T.astype(bf)
        xt[C, :] = 1.0
        wq = np.zeros((KS * P, 3 * QK), dtype=bf)
        for s in range(3):  # q, k, v sections
            cols = qkv_w[:, s * C + hs * D: s * C + (hs + HL) * D]
            wq[0:C, s * QK:(s + 1) * QK] = cols.astype(bf)
        wq[C, 0:QK] = qkv_b[hs * D:(hs + HL) * D].astype(bf)
        wq[C, QK:2 * QK] = qkv_b[C + hs * D: C + (hs + HL) * D].astype(bf)
        qk_bias = np.concatenate([
            qkv_b[hs * D:(hs + HL) * D], qkv_b[C + hs * D: C + (hs + HL) * D]
        ]).astype(np.float32)
        in_maps.append({"xt": xt, "wq": wq,
                        "bias_qk": np.ascontiguousarray(qk_bias.reshape(6, P).T)})
    return in_maps


def kernel(x, qkv_w, qkv_b, proj_w, proj_b):
    from concourse.bass_utils import run_bass_kernel_spmd

    x = np.asarray(x, dtype=np.float32)
    qkv_w = np.asarray(qkv_w, dtype=np.float32)
    qkv_b = np.asarray(qkv_b, dtype=np.float32)
    proj_w = np.asarray(proj_w, dtype=np.float32)
    proj_b = np.asarray(proj_b, dtype=np.float32)

    if "nc" not in _cache:
        _cache["nc"] = _build()
    nc = _cache["nc"]

    bf = ml_dtypes.bfloat16
    in_maps = _prep_inputs(x, qkv_w, qkv_b)
    for c in range(8):
        hs = (c % 2) * HL
        in_maps[c]["wp"] = proj_w[hs * D:(hs + HL) * D, :].astype(bf)

    res = run_bass_kernel_spmd(nc, in_maps, core_ids=list(range(8)))
    parts = [res.results[c]["out"].astype(np.float32) for c in range(8)]

    # v-bias contribution (exact, f32) + proj bias, added once per batch
    const_row = qkv_b[2 * C:] @ proj_w + proj_b
    out = np.empty((B, N, C), dtype=np.float32)
    for b in range(B):
        out[b] = parts[2 * b] + parts[2 * b + 1] + const_row
    return out
